# revision 6
# baseline (speedup 1.0000x reference)
"""Trainium2 Bass kernel: conditional logistic regression (segmented softmax).

Problem: X [N=4194304, 64] fp32, sorted segment_ids [N] (65536 segments,
avg 64 rows), W [1,64], b [1].
  logits = X @ W.T + b ; out = segmented_softmax(logits, segment_ids)

Strategy (8 cores, data-parallel over N):
  - Each core owns N/8 = 524288 consecutive rows, split into 128 spans of
    S = 4096 rows (one span per SBUF partition).
  - Overlap-pad trick: each span processes a window [span_start - PAD,
    span_end + PAD) with PAD >= max segment length, so every segment whose
    rows intersect the span core is fully contained in the window. Pad rows
    are computed redundantly and discarded; no cross-partition traffic.
  - Host prep: X is cast to fp16 (halves HBM traffic; logit error ~1e-3,
    well under tolerance) and pre-gathered into PE-stationary layout:
    xtc[r*64+d, j*128+m] = Xwin[m*S + 2j + r, d]. Each 128x128 fp16 slab is
    the stationary operand of one matmul against w2 [128, 2] (two
    half-partition copies of W), producing logits for 2 positions x 128
    spans directly in scan layout [span, pos] in PSUM. No on-device
    transposes, no vector-engine matvec. b dropped (cancels in softmax).
  - Segment-boundary mask kbx (kbx[t]=1 iff ids[t]==ids[t-1]) is computed
    on host, shipped as fp32.
  - exp on ACT (PSUM bank -> SBUF E). Segmented sums via DVE
    tensor_tensor_scan: forward masked-sum scan, backward propagate scan,
    fast reciprocal, multiply. Output written per sub-window so scans
    overlap the matmul/DMA stream.
"""

import numpy as np

import concourse.bass as bass
import concourse.tile as tile
from concourse import mybir
from concourse.alu_op_type import AluOpType

F16 = mybir.dt.float16
F32 = mybir.dt.float32

# Full problem constants
N_FULL = 4194304
D = 64
N_CORES = 8
SPANS = 128


def _rev(ap):
    """Reverse an AP along its (last) free dim."""
    return ap[:, ::-1]


def _split_multi_waits(nc):
    """Hoist extra sync waits into standalone EventSemaphore instructions.

    Engine compute/DMA instruction encodings only support a single sync-wait
    slot (walrus: "Too many sync wait commands"); standalone waits execute on
    the same engine sequencer in program order, so semantics are unchanged.
    """
    exempt = ()
    n = 0
    for f in nc.m.functions:
        for blk in f.blocks:
            insts = list(blk.instructions)
            out = []
            for ins in insts:
                si = ins.sync_info
                if (
                    si is not None
                    and si.on_wait
                    and len(si.on_wait) > 1
                    and type(ins).__name__ not in exempt
                ):
                    waits = list(si.on_wait)
                    for w in waits[:-1]:
                        es = mybir.InstEventSemaphore(
                            name=f"W-split-{n}", ins=[], outs=[]
                        )
                        n += 1
                        es.engine = ins.engine
                        es.sync_info = mybir.SyncInfo(on_wait=[w], on_update=[])
                        nc.inst_map[es.name] = es
                        out.append(es)
                    ins.sync_info = mybir.SyncInfo(
                        on_wait=[waits[-1]], on_update=list(si.on_update)
                    )
                out.append(ins)
            if len(out) != len(insts):
                blk.instructions = out
    return n


def build_nc(S, PAD, trn=None):
    L = S + 2 * PAD          # per-span window length (rows)
    Lh = L // 2              # row-pairs per span == matmuls per core
    TPC = 32                 # stationary tiles (row-pairs) per DMA chunk
    CF = TPC * 128           # fp16 elems per partition per chunk
    assert Lh % TPC == 0
    nchunks = Lh // TPC
    PPB = 128                # row-pairs per exp block (256 f32 cols;
                             # PSUM tile stays bank-sized, half used, so ACT
                             # never reads a bank PE is writing)
    F = Lh * 128             # xtc free length per partition
    H = 16
    Sh = S // H

    nc = bass.Bass(trn, target_bir_lowering=False)
    xtc = nc.dram_tensor("xtc", [128 * F], F16, kind="ExternalInput")
    kbxd = nc.dram_tensor("kbx", [SPANS * (L + 1)], F16, kind="ExternalInput")
    w2d = nc.dram_tensor("w2", [128, 2], F16, kind="ExternalInput")
    out = nc.dram_tensor("out", [SPANS * S], F16, kind="ExternalOutput")

    # Sub-window softmax pipelines: split the core [PAD, PAD+S) into H
    # parts; each part's segments live within its part +- PAD, so each
    # sub-window [a,b) runs its own scan chain as soon as its E columns
    # exist (overlap-pad trick applied recursively).
    sub = []
    for h in range(H):
        a = h * Sh
        b = min(L, (h + 1) * Sh + 2 * PAD)
        sub.append((a, b))

    with tile.TileContext(nc) as tc:
        with (
            tc.tile_pool(name="xin", bufs=5) as xin_pool,
            tc.tile_pool(name="pps", bufs=6, space="PSUM") as pps_pool,
            tc.tile_pool(name="ev", bufs=2) as ev_pool,
            tc.tile_pool(name="rt", bufs=2) as rt_pool,
            tc.tile_pool(name="big", bufs=1) as big,
        ):
            w2_sb = big.tile([128, 2], F16, tag="w2")
            nc.scalar.dma_start(out=w2_sb[:, :], in_=w2d[:, :])
            kbx = big.tile([SPANS, L + 1], F16, tag="kbx")
            nc.scalar.dma_start(
                out=kbx[:, :],
                in_=bass.AP(
                    tensor=kbxd, offset=0, ap=[[L + 1, SPANS], [1, L + 1]]
                ),
            )
            E = big.tile([SPANS, L], F32, tag="E")
            s_run = big.tile([SPANS, L], F32, tag="srun")

            def emit_subwindow(h):
                a, b = sub[h]
                w = b - a
                assert w <= 4095
                # forward masked-sum scan: s[t] = kbx[t]*s[t-1] + E[t]
                nc.vector.tensor_tensor_scan(
                    out=s_run[:, a:b], data0=kbx[:, a:b], data1=E[:, a:b],
                    initial=0.0, op0=AluOpType.mult, op1=AluOpType.add,
                )
                # ev[t] = s[t] * (1 - kbx[t+1]): segment total at segment
                # end positions, 0 elsewhere
                ev = ev_pool.tile([SPANS, w], F32, tag="ev")
                nc.vector.tensor_tensor(
                    out=ev[:, :], in0=s_run[:, a:b], in1=kbx[:, a + 1 : b + 1],
                    op=AluOpType.mult,
                )
                nc.vector.tensor_tensor(
                    out=ev[:, :], in0=s_run[:, a:b], in1=ev[:, :],
                    op=AluOpType.subtract,
                )
                # backward propagate: spread each segment total over its rows
                nc.vector.tensor_tensor_scan(
                    out=_rev(s_run[:, a:b]), data0=_rev(kbx[:, a + 1 : b + 1]),
                    data1=_rev(ev[:, :]), initial=0.0,
                    op0=AluOpType.mult, op1=AluOpType.add,
                )
                c0, c1 = PAD + h * Sh, PAD + (h + 1) * Sh
                rt = rt_pool.tile([SPANS, Sh], F32, tag="rt")
                nc.vector.reciprocal(out=rt[:, :], in_=s_run[:, c0:c1])
                r16 = rt_pool.tile([SPANS, Sh], F16, tag="r16")
                nc.vector.tensor_tensor(
                    out=r16[:, :], in0=E[:, c0:c1], in1=rt[:, :],
                    op=AluOpType.mult,
                )
                nc.scalar.dma_start(
                    out=bass.AP(
                        tensor=out, offset=h * Sh, ap=[[S, SPANS], [1, Sh]]
                    ),
                    in_=r16[:, :],
                )

            next_sub = 0
            PS = None
            for j in range(nchunks):
                xt = xin_pool.tile([128, CF], F16, tag="xt")
                nc.sync.dma_start(
                    out=xt[:, :],
                    in_=bass.AP(
                        tensor=xtc, offset=j * CF, ap=[[F, 128], [1, CF]]
                    ),
                )
                for t in range(TPC):
                    g = j * TPC + t
                    if g % PPB == 0:
                        PS = pps_pool.tile([128, 512], F32, tag="ps")
                    col = 2 * (g % PPB)
                    nc.tensor.matmul(
                        PS[:, col : col + 2],
                        lhsT=xt[:, t * 128 : (t + 1) * 128],
                        rhs=w2_sb[:, :], start=True, stop=True,
                    )
                    if (g + 1) % PPB == 0 or g == Lh - 1:
                        b0 = (g // PPB) * PPB
                        nc.scalar.activation(
                            out=E[:, 2 * b0 : 2 * (g + 1)],
                            in_=PS[:, 0 : 2 * (g + 1) - 2 * b0],
                            func=mybir.ActivationFunctionType.Exp,
                        )
                        while next_sub < H and sub[next_sub][1] <= 2 * (g + 1):
                            emit_subwindow(next_sub)
                            next_sub += 1
            assert next_sub == H
    _split_multi_waits(nc)
    return nc


def _prep_host(X, segment_ids, W, S, PAD, n_cores):
    N = X.shape[0]
    Nc = SPANS * S
    assert Nc * n_cores == N
    L = S + 2 * PAD
    Lh = L // 2
    Sh2 = S // 2

    X16 = np.zeros((N + 2 * PAD, D), np.float16)
    X16[PAD : N + PAD] = np.asarray(X, np.float32)
    Bf = X16.reshape(-1, 128)  # row u = padded rows (2u, 2u+1) flattened

    ids = np.asarray(segment_ids).astype(np.int64)
    idsp = np.full(N + 2 * PAD, -1, np.int64)
    idsp[PAD : N + PAD] = ids
    eq = np.zeros(N + 2 * PAD + 1, np.float32)
    eq[1 : N + 2 * PAD] = (idsp[1:] == idsp[:-1]).astype(np.float32)

    Wf = np.asarray(W, np.float32).ravel()
    w2 = np.zeros((128, 2), np.float16)
    w2[0:64, 0] = Wf
    w2[64:128, 1] = Wf

    m_off_u = np.arange(SPANS) * Sh2
    t_idx = np.arange(L + 1)
    JB = 64
    in_maps = []
    for c in range(n_cores):
        base_u = c * (Nc // 2)
        xtc = np.empty((128, Lh, 128), np.float16)
        for j0 in range(0, Lh, JB):
            jb = min(JB, Lh - j0)
            rows = base_u + j0 + np.arange(jb)[:, None] + m_off_u[None, :]
            blk = Bf[rows]  # (jb, 128 spans, 128 p) gather, cache-friendly
            xtc[:, j0 : j0 + jb, :] = blk.transpose(2, 0, 1)
        kb = eq[(c * Nc + np.arange(SPANS) * S)[:, None] + t_idx[None, :]].copy()
        kb[:, 0] = 0.0   # window start is a forced segment cut
        kb[:, L] = 0.0   # window end likewise (backward-scan reset)
        in_maps.append(
            {"xtc": xtc.reshape(-1),
             "kbx": np.ascontiguousarray(kb).reshape(-1).astype(np.float16),
             "w2": w2}
        )
    return in_maps


def kernel(X, segment_ids, W, b, _return_results=False, _trace=False):
    from concourse import bass_utils

    X = np.asarray(X)
    N = X.shape[0]
    assert N == N_FULL, f"kernel hardcoded for N={N_FULL}, got {N}"
    S = N // (N_CORES * SPANS)
    m = _max_seg_len(segment_ids)
    PAD = max(64, int(np.ceil(m / 64.0)) * 64)

    nc = build_nc(S, PAD)
    in_maps = _prep_host(X, segment_ids, W, S, PAD, N_CORES)
    res = bass_utils.run_bass_kernel_spmd(
        nc, in_maps, core_ids=list(range(N_CORES)), trace=_trace
    )
    out = np.concatenate([r["out"] for r in res.results]).astype(np.float32)
    if _return_results:
        return out, res
    return out


def _max_seg_len(segment_ids):
    ids = np.asarray(segment_ids).astype(np.int64)
    change = np.flatnonzero(np.diff(ids) != 0)
    starts = np.concatenate([[0], change + 1])
    ends = np.concatenate([change + 1, [len(ids)]])
    return int((ends - starts).max())


# revision 7
# speedup vs baseline: 1.0941x; 1.0941x over previous
"""Trainium2 Bass kernel: conditional logistic regression (segmented softmax).

Problem: X [N=4194304, 64] fp32, sorted segment_ids [N] (65536 segments,
avg 64 rows), W [1,64], b [1].
  logits = X @ W.T + b ; out = segmented_softmax(logits, segment_ids)

Strategy (8 cores, data-parallel over N):
  - Each core owns N/8 = 524288 consecutive rows, split into 128 spans of
    S = 4096 rows (one span per SBUF partition).
  - Overlap-pad trick: each span processes a window [span_start - PAD,
    span_end + PAD) with PAD >= max segment length, so every segment whose
    rows intersect the span core is fully contained in the window. Pad rows
    are computed redundantly and discarded; no cross-partition traffic.
  - Host prep: X is cast to fp16 (halves HBM traffic; logit error ~1e-3,
    well under tolerance) and pre-gathered into PE-stationary layout:
    xtc[r*64+d, j*128+m] = Xwin[m*S + 2j + r, d]. Each 128x128 fp16 slab is
    the stationary operand of one matmul against w2 [128, 2] (two
    half-partition copies of W), producing logits for 2 positions x 128
    spans directly in scan layout [span, pos] in PSUM. No on-device
    transposes, no vector-engine matvec. b dropped (cancels in softmax).
  - Segment-boundary mask kbx (kbx[t]=1 iff ids[t]==ids[t-1]) is computed
    on host, shipped as fp32.
  - exp on ACT (PSUM bank -> SBUF E). Segmented sums via DVE
    tensor_tensor_scan: forward masked-sum scan, backward propagate scan,
    fast reciprocal, multiply. Output written per sub-window so scans
    overlap the matmul/DMA stream.
"""

import numpy as np

import concourse.bass as bass
import concourse.tile as tile
from concourse import mybir
from concourse.alu_op_type import AluOpType

F16 = mybir.dt.float16
F32 = mybir.dt.float32

# Full problem constants
N_FULL = 4194304
D = 64
N_CORES = 8
SPANS = 128


def _rev(ap):
    """Reverse an AP along its (last) free dim."""
    return ap[:, ::-1]


def _split_multi_waits(nc):
    """Hoist extra sync waits into standalone EventSemaphore instructions.

    Engine compute/DMA instruction encodings only support a single sync-wait
    slot (walrus: "Too many sync wait commands"); standalone waits execute on
    the same engine sequencer in program order, so semantics are unchanged.
    """
    exempt = ()
    n = 0
    for f in nc.m.functions:
        for blk in f.blocks:
            insts = list(blk.instructions)
            out = []
            for ins in insts:
                si = ins.sync_info
                if (
                    si is not None
                    and si.on_wait
                    and len(si.on_wait) > 1
                    and type(ins).__name__ not in exempt
                ):
                    waits = list(si.on_wait)
                    for w in waits[:-1]:
                        es = mybir.InstEventSemaphore(
                            name=f"W-split-{n}", ins=[], outs=[]
                        )
                        n += 1
                        es.engine = ins.engine
                        es.sync_info = mybir.SyncInfo(on_wait=[w], on_update=[])
                        nc.inst_map[es.name] = es
                        out.append(es)
                    ins.sync_info = mybir.SyncInfo(
                        on_wait=[waits[-1]], on_update=list(si.on_update)
                    )
                out.append(ins)
            if len(out) != len(insts):
                blk.instructions = out
    return n


def build_nc(S, PAD, trn=None):
    L = S + 2 * PAD          # per-span window length (rows)
    Lh = L // 2              # row-pairs per span == matmuls per core
    TPC = 32                 # stationary tiles (row-pairs) per DMA chunk
    CF = TPC * 128           # fp16 elems per partition per chunk
    assert Lh % TPC == 0
    nchunks = Lh // TPC
    PPB = 128                # row-pairs per exp block (256 f32 cols;
                             # PSUM tile stays bank-sized, half used, so ACT
                             # never reads a bank PE is writing)
    F = Lh * 128             # xtc free length per partition
    H = 16
    Sh = S // H

    nc = bass.Bass(trn, target_bir_lowering=False)
    xtc = nc.dram_tensor("xtc", [128 * F], F16, kind="ExternalInput")
    kbxd = nc.dram_tensor("kbx", [SPANS * (L + 1)], F16, kind="ExternalInput")
    w2d = nc.dram_tensor("w2", [128, 2], F16, kind="ExternalInput")
    out = nc.dram_tensor("out", [SPANS * S], F16, kind="ExternalOutput")

    # Sub-window softmax pipelines: split the core [PAD, PAD+S) into H
    # parts; each part's segments live within its part +- PAD, so each
    # sub-window [a,b) runs its own scan chain as soon as its E columns
    # exist (overlap-pad trick applied recursively).
    sub = []
    for h in range(H):
        a = h * Sh
        b = min(L, (h + 1) * Sh + 2 * PAD)
        sub.append((a, b))

    with tile.TileContext(nc) as tc:
        with (
            tc.tile_pool(name="xin", bufs=5) as xin_pool,
            tc.tile_pool(name="pps", bufs=8, space="PSUM") as pps_pool,
            tc.tile_pool(name="ev", bufs=2) as ev_pool,
            tc.tile_pool(name="rt", bufs=2) as rt_pool,
            tc.tile_pool(name="r16", bufs=3) as r16_pool,
            tc.tile_pool(name="big", bufs=1) as big,
        ):
            w2_sb = big.tile([128, 2], F16, tag="w2")
            nc.scalar.dma_start(out=w2_sb[:, :], in_=w2d[:, :])
            kbx = big.tile([SPANS, L + 1], F16, tag="kbx")
            nc.scalar.dma_start(
                out=kbx[:, :],
                in_=bass.AP(
                    tensor=kbxd, offset=0, ap=[[L + 1, SPANS], [1, L + 1]]
                ),
            )
            E = big.tile([SPANS, L], F32, tag="E")
            s_run = big.tile([SPANS, L], F32, tag="srun")

            def emit_subwindow(h):
                a, b = sub[h]
                w = b - a
                assert w <= 4095
                # forward masked-sum scan: s[t] = kbx[t]*s[t-1] + E[t]
                nc.vector.tensor_tensor_scan(
                    out=s_run[:, a:b], data0=kbx[:, a:b], data1=E[:, a:b],
                    initial=0.0, op0=AluOpType.mult, op1=AluOpType.add,
                )
                # ev[t] = s[t] * (1 - kbx[t+1]): segment total at segment
                # end positions, 0 elsewhere
                ev = ev_pool.tile([SPANS, w], F32, tag="ev")
                nc.vector.tensor_tensor(
                    out=ev[:, :], in0=s_run[:, a:b], in1=kbx[:, a + 1 : b + 1],
                    op=AluOpType.mult,
                )
                nc.vector.tensor_tensor(
                    out=ev[:, :], in0=s_run[:, a:b], in1=ev[:, :],
                    op=AluOpType.subtract,
                )
                # backward propagate: spread each segment total over its rows
                nc.vector.tensor_tensor_scan(
                    out=_rev(s_run[:, a:b]), data0=_rev(kbx[:, a + 1 : b + 1]),
                    data1=_rev(ev[:, :]), initial=0.0,
                    op0=AluOpType.mult, op1=AluOpType.add,
                )
                c0, c1 = PAD + h * Sh, PAD + (h + 1) * Sh
                rt = rt_pool.tile([SPANS, Sh], F32, tag="rt")
                nc.vector.reciprocal(out=rt[:, :], in_=s_run[:, c0:c1])
                r16 = r16_pool.tile([SPANS, Sh], F16, tag="r16")
                nc.vector.tensor_tensor(
                    out=r16[:, :], in0=E[:, c0:c1], in1=rt[:, :],
                    op=AluOpType.mult,
                )
                deferred_out.append((h, r16))

            def flush_out(h, r16):
                nc.scalar.dma_start(
                    out=bass.AP(
                        tensor=out, offset=h * Sh, ap=[[S, SPANS], [1, Sh]]
                    ),
                    in_=r16[:, :],
                )

            next_sub = 0
            deferred_out = []
            # (emit_chunk, h, tile): out-DMA issued DEFER chunks after its
            # scan chain starts, so the ACT ring never stalls behind DVE
            DEFER = 3
            pending = []
            PS = None
            for j in range(nchunks):
                while pending and pending[0][0] <= j:
                    _, h, r16 = pending.pop(0)
                    flush_out(h, r16)
                xt = xin_pool.tile([128, CF], F16, tag="xt")
                nc.sync.dma_start(
                    out=xt[:, :],
                    in_=bass.AP(
                        tensor=xtc, offset=j * CF, ap=[[F, 128], [1, CF]]
                    ),
                )
                for t in range(TPC):
                    g = j * TPC + t
                    if g % PPB == 0:
                        PS = pps_pool.tile([128, 512], F32, tag="ps")
                    col = 2 * (g % PPB)
                    nc.tensor.matmul(
                        PS[:, col : col + 2],
                        lhsT=xt[:, t * 128 : (t + 1) * 128],
                        rhs=w2_sb[:, :], start=True, stop=True,
                    )
                    if (g + 1) % PPB == 0 or g == Lh - 1:
                        b0 = (g // PPB) * PPB
                        nc.scalar.activation(
                            out=E[:, 2 * b0 : 2 * (g + 1)],
                            in_=PS[:, 0 : 2 * (g + 1) - 2 * b0],
                            func=mybir.ActivationFunctionType.Exp,
                        )
                        while next_sub < H and sub[next_sub][1] <= 2 * (g + 1):
                            emit_subwindow(next_sub)
                            h2, r16_2 = deferred_out.pop()
                            pending.append((j + DEFER, h2, r16_2))
                            next_sub += 1
            for _, h2, r16_2 in pending:
                flush_out(h2, r16_2)
            assert next_sub == H and not deferred_out
    _split_multi_waits(nc)
    return nc


def _prep_host(X, segment_ids, W, S, PAD, n_cores):
    N = X.shape[0]
    Nc = SPANS * S
    assert Nc * n_cores == N
    L = S + 2 * PAD
    Lh = L // 2
    Sh2 = S // 2

    X16 = np.zeros((N + 2 * PAD, D), np.float16)
    X16[PAD : N + PAD] = np.asarray(X, np.float32)
    Bf = X16.reshape(-1, 128)  # row u = padded rows (2u, 2u+1) flattened

    ids = np.asarray(segment_ids).astype(np.int64)
    idsp = np.full(N + 2 * PAD, -1, np.int64)
    idsp[PAD : N + PAD] = ids
    eq = np.zeros(N + 2 * PAD + 1, np.float32)
    eq[1 : N + 2 * PAD] = (idsp[1:] == idsp[:-1]).astype(np.float32)

    Wf = np.asarray(W, np.float32).ravel()
    w2 = np.zeros((128, 2), np.float16)
    w2[0:64, 0] = Wf
    w2[64:128, 1] = Wf

    m_off_u = np.arange(SPANS) * Sh2
    t_idx = np.arange(L + 1)
    JB = 64
    in_maps = []
    for c in range(n_cores):
        base_u = c * (Nc // 2)
        xtc = np.empty((128, Lh, 128), np.float16)
        for j0 in range(0, Lh, JB):
            jb = min(JB, Lh - j0)
            rows = base_u + j0 + np.arange(jb)[:, None] + m_off_u[None, :]
            blk = Bf[rows]  # (jb, 128 spans, 128 p) gather, cache-friendly
            xtc[:, j0 : j0 + jb, :] = blk.transpose(2, 0, 1)
        kb = eq[(c * Nc + np.arange(SPANS) * S)[:, None] + t_idx[None, :]].copy()
        kb[:, 0] = 0.0   # window start is a forced segment cut
        kb[:, L] = 0.0   # window end likewise (backward-scan reset)
        in_maps.append(
            {"xtc": xtc.reshape(-1),
             "kbx": np.ascontiguousarray(kb).reshape(-1).astype(np.float16),
             "w2": w2}
        )
    return in_maps


def kernel(X, segment_ids, W, b, _return_results=False, _trace=False):
    from concourse import bass_utils

    X = np.asarray(X)
    N = X.shape[0]
    assert N == N_FULL, f"kernel hardcoded for N={N_FULL}, got {N}"
    S = N // (N_CORES * SPANS)
    m = _max_seg_len(segment_ids)
    PAD = max(64, int(np.ceil(m / 64.0)) * 64)

    nc = build_nc(S, PAD)
    in_maps = _prep_host(X, segment_ids, W, S, PAD, N_CORES)
    res = bass_utils.run_bass_kernel_spmd(
        nc, in_maps, core_ids=list(range(N_CORES)), trace=_trace
    )
    out = np.concatenate([r["out"] for r in res.results]).astype(np.float32)
    if _return_results:
        return out, res
    return out


def _max_seg_len(segment_ids):
    ids = np.asarray(segment_ids).astype(np.int64)
    change = np.flatnonzero(np.diff(ids) != 0)
    starts = np.concatenate([[0], change + 1])
    ends = np.concatenate([change + 1, [len(ids)]])
    return int((ends - starts).max())
